# revision 14
# baseline (speedup 1.0000x reference)
"""NVFP4-style activation quantizer on 8 TRN2 NeuronCores (raw bass).

Single-pass over HBM: x streams in once; an fp16 copy (x16) of the whole
shard stays resident in SBUF, so the post-AllReduce quantize pass re-reads
nothing from HBM (67MB traffic/core instead of 100MB for two-pass).

Reference semantics (per 16-element block, fp32):
    s_t  = max|x| / (6*448)                      (global, needs all-reduce)
    m_b  = max|x| over block
    s_b  = fp8_e4m3_roundtrip(6 * s_t / m_b)
    out  = sign(x) * fp4_121(|x|/s_t * s_b) / s_b * s_t

Device algorithm per element (signed, select-free), fp16 magic rounding:
    y16 = fp16(x16 * c)        c = s_b / s_t  (per block, fp32)
    p   = (bits(y16) & 0x7C00) + 0x2600        (u16 ops)
    B   = max(p, 0x6200)                       (= 768*max(2^e,1) as fp16)
    t   = fp16(y16 + B)        (fp16 RNE add rounds y to the fp4 grid)
    nq  = B - t                (exact; = -fp4_121(y)*sign)
    out = nq * nic             nic = -s_t/s_b  (per block, fp16)

Measured rel_err vs the fp32 reference on seed-0 data: 8.2e-3.

Engine split (from measured per-op costs: DVE TT 1.27us/[P,1024] at 1x,
DVE TS 0.60us at 2x, Pool TT 2.45us, no DVE fast mode for TT):
  SYNC   input DMAs + collective staging + output DMAs
  ACT    fp16 conversion of x into resident x16 (nothing else)
  DVE    pass-A block maxes, per-block scales, the u16/fp16 rounding ops
         (and/add+max/t/nq) for all tiles, plus a small share of the two
         per-block broadcast multiplies
  POOL   AllReduce (warmed up) + most broadcast multiplies (y16 and the
         final nq*nic), balancing DVE
"""

import numpy as np

FULL_SHAPE = (4, 4096, 4096)
N_CORES = 8
P = 128
TOTAL = 4 * 4096 * 4096
L = TOTAL // (N_CORES * P)   # 65536 elements per partition per core
NBLK = L // 16               # 4096 blocks per partition

M16 = 0x7C00                 # fp16 exponent mask
A16 = 0x2600                 # exponent +9, set mantissa bit -> *768
MN16 = 0x6200                # bits of fp16 768.0


def build_nc(n_cores=N_CORES, FA=2048, FB=512, n_xa=3, n_o=4,
             n_chunks=8, yp_mod=27, sp_mod=27):
    """yp_mod/sp_mod: of every 32 tiles, how many get their y16 / final
    multiply executed on Pool (the rest stay on DVE)."""
    from contextlib import ExitStack

    import concourse.bass as bass
    from concourse import mybir

    f32 = mybir.dt.float32
    f16 = mybir.dt.float16
    u16 = mybir.dt.uint16
    f8 = mybir.dt.float8e4

    TA = L // FA                 # pass-A tiles (32)
    TB = L // FB                 # pass-B tiles (64)
    fbB = FB // 16               # blocks per pass-B tile (64)
    QN = NBLK // n_chunks        # blocks per scale chunk (512)
    TPQ = TB // n_chunks         # pass-B tiles per chunk (8)
    assert L % FA == 0 and L % FB == 0 and NBLK % n_chunks == 0
    assert TB % n_chunks == 0 and TPQ % 2 == 0

    def y_pool(t):
        return (t % 32) < yp_mod

    def s6_pool(t):
        return (t % 32) < sp_mod

    # pool cumulative tag schedule (DVE traced first, needs these).
    # pool stream: warmup memset (+1), then per tile t: [y16(t) if pool],
    # [step6(t-1) if pool]; trailing step6(TB-1).
    ptagy = [0] * TB
    ptag6 = [0] * TB
    pc = 1
    for t in range(TB):
        if y_pool(t):
            pc += 1
        ptagy[t] = pc
        if t >= 2 and s6_pool(t - 2):
            pc += 1
            ptag6[t - 2] = pc
    for u in (TB - 2, TB - 1):
        if s6_pool(u):
            pc += 1
            ptag6[u] = pc

    nc = bass.Bass(num_devices=n_cores, debug=False)
    x_ext = nc.declare_dram_parameter("x", [P, L], f32, isOutput=False)
    out_ext = nc.declare_dram_parameter("out", [P, L], f32, isOutput=True)
    cc_in = nc.dram_tensor("cc_in", [1, 128], f32)
    cc_out = nc.dram_tensor("cc_out", [1, 128], f32, addr_space="Shared")
    cc_warm_in = nc.dram_tensor("cc_warm_in", [1, 128], f32)
    cc_warm_out = nc.dram_tensor("cc_warm_out", [1, 128], f32,
                                 addr_space="Shared")

    with ExitStack() as ctx:
        def sem(name):
            return ctx.enter_context(nc.semaphore(name))

        def sbuf(name, shape, dt=f32):
            return ctx.enter_context(nc.sbuf_tensor(name, shape, dt))

        s_xa = [sem(f"s_xa{i}") for i in range(n_xa)]   # input DMAs   (+16)
        s_ob = [sem(f"s_ob{i}") for i in range(n_o)]    # output DMAs  (+16)
        s_cdma = sem("s_cdma")   # collective staging DMAs             (+16)
        s_act = sem("s_act")     # ACT x16 conversions                 (+1)
        s_dve = sem("s_dve")     # tagged DVE ops                      (+1)
        s_pool = sem("s_pool")   # pool ops                            (+1)
        s_cc = sem("s_cc")       # collectives                         (+1)
        s_warm = sem("s_warm")   # warm-up staging dma                 (+16)

        xa = [sbuf(f"xa{i}", [P, FA]) for i in range(n_xa)]
        x16 = sbuf("x16", [P, L], f16)        # resident fp16 shard (128KB)
        m_t = sbuf("m_t", [P, NBLK])          # block max m -> c
        r_t = sbuf("r_t", [P, NBLK])          # rm=1/m (AR window) -> inv
        nic2 = [sbuf(f"nic2_{i}", [P, QN], f16) for i in range(2)]
        f8_q = sbuf("f8_q", [P, QN], f8)      # chunk temps
        sb_q = sbuf("sb_q", [P, QN])
        t1_q = sbuf("t1_q", [P, QN])
        yb = [sbuf(f"yb{i}", [P, FB], f16) for i in range(4)]
        pb = [sbuf(f"pb{i}", [P, FB], u16) for i in range(4)]
        ob = [sbuf(f"ob{i}", [P, FB]) for i in range(n_o)]
        gall = sbuf("gall", [P, 128])
        mx_t = sbuf("mx_t", [P, 1])
        mxp_t = sbuf("mxp_t", [P, 1])
        g128 = sbuf("g128", [P, 1])
        st_t = sbuf("st_t", [P, 1])
        rt_t = sbuf("rt_t", [P, 1])
        nst_t = sbuf("nst_t", [P, 1])

        tagA = [0] * TA
        tag_y = [0] * TB         # DVE y16 tags (when on DVE)
        tag_nq = [0] * TB        # nq (last rounding op, yb slot)
        tag6 = [0] * TB          # DVE final-multiply tags (when on DVE)
        K_mx = [0]
        qc_tag = [0] * n_chunks  # c chunk ready (DVE tag)
        qn_tag = [0] * n_chunks  # nic16 chunk ready (DVE tag)

        def b3(ap):
            return ap.rearrange("p (b s) -> p b s", s=16)

        def bc(ap_slice, nb):
            return ap_slice.unsqueeze(-1).broadcast_to([P, nb, 16])

        def y_args(t):
            return dict(
                out=b3(yb[t % 4][:]),
                in0=b3(x16[:, t * FB:(t + 1) * FB]),
                in1=bc(m_t[:, t * fbB:(t + 1) * fbB], fbB),
            )

        def s6_args(t):
            q = t // TPQ
            w = t % TPQ
            return dict(
                out=b3(ob[t % n_o][:]),
                in0=b3(yb[t % 4][:]),
                in1=bc(nic2[q % 2][:, w * fbB:(w + 1) * fbB], fbB),
            )

        with nc.Block() as block:

            @block.vector
            def _(dve):
                cnt = 0

                def tag(ins):
                    nonlocal cnt
                    ins.then_inc(s_dve)
                    cnt += 1
                    return cnt

                # ---- pass A: per-block abs max ----
                # shard max accumulates incrementally every 8 tiles so the
                # AllReduce staging starts ~1.5us after the last reduce
                # instead of after a full 4096-wide reduce
                fbA = FA // 16
                for t in range(TA):
                    dve.wait_ge(s_xa[t % n_xa], 16 * (t // n_xa + 1))
                    tagA[t] = tag(dve.tensor_reduce(
                        out=m_t[:, t * fbA:(t + 1) * fbA],
                        in_=b3(xa[t % n_xa][:]),
                        axis=mybir.AxisListType.X,
                        op=mybir.AluOpType.max,
                        apply_absolute_value=True,
                    ))
                    if (t + 1) % 8 == 0:
                        c0 = (t - 7) * fbA
                        c1 = (t + 1) * fbA
                        dve.wait_ge(s_dve, tagA[t])
                        if t == 7:
                            km = tag(dve.tensor_reduce(
                                out=mx_t[:], in_=m_t[:, c0:c1],
                                axis=mybir.AxisListType.X,
                                op=mybir.AluOpType.max))
                        else:
                            kc = tag(dve.tensor_reduce(
                                out=mxp_t[:], in_=m_t[:, c0:c1],
                                axis=mybir.AxisListType.X,
                                op=mybir.AluOpType.max))
                            dve.wait_ge(s_dve, kc)
                            km = tag(dve.tensor_tensor(
                                mx_t[:], mx_t[:], mxp_t[:],
                                op=mybir.AluOpType.max))
                        dve.wait_ge(s_dve, km)
                K_mx[0] = km
                # rm = 1/m for the whole shard, hidden in the AllReduce window
                h = NBLK // 2
                k_rm0 = tag(dve.reciprocal(r_t[:, 0:h], m_t[:, 0:h]))
                k_rm1 = tag(dve.reciprocal(r_t[:, h:NBLK], m_t[:, h:NBLK]))

                # ---- post-AllReduce scalars ----
                dve.wait_ge(s_cdma, 32)        # gall loaded (bcast DMA)
                k1 = tag(dve.tensor_reduce(
                    out=g128[:], in_=gall[:], axis=mybir.AxisListType.X,
                    op=mybir.AluOpType.max))
                dve.wait_ge(s_dve, k1)
                k2 = tag(dve.tensor_scalar(
                    st_t[:], g128[:], 1.0 / 2688.0, None,
                    op0=mybir.AluOpType.mult))
                dve.wait_ge(s_dve, k2)
                k3 = tag(dve.reciprocal(rt_t[:], st_t[:]))
                k4 = tag(dve.tensor_scalar(
                    nst_t[:], st_t[:], -1.0, None, op0=mybir.AluOpType.mult))
                dve.wait_ge(s_dve, k4)
                dve.wait_ge(s_dve, k3)
                dve.wait_ge(s_dve, k_rm0)
                dve.wait_ge(s_dve, k_rm1)
                seen_act = [False]

                # ---- per-chunk scales + pass-B tiles ----
                for q in range(n_chunks):
                    sl = slice(q * QN, (q + 1) * QN)
                    a = tag(dve.tensor_scalar(      # inv = rm*st*6 in-place
                        r_t[:, sl], r_t[:, sl], st_t[:], 6.0,
                        op0=mybir.AluOpType.mult, op1=mybir.AluOpType.mult))
                    dve.wait_ge(s_dve, a)
                    b_ = tag(dve.tensor_copy(f8_q[:], r_t[:, sl]))
                    dve.wait_ge(s_dve, b_)
                    c_ = tag(dve.tensor_copy(sb_q[:], f8_q[:]))   # s_b
                    dve.wait_ge(s_dve, c_)
                    d_ = tag(dve.tensor_scalar(     # c = s_b/s_t -> m_t
                        m_t[:, sl], sb_q[:], rt_t[:], None,
                        op0=mybir.AluOpType.mult))
                    e_ = tag(dve.reciprocal(t1_q[:], sb_q[:]))    # 1/s_b
                    dve.wait_ge(s_dve, e_)
                    if q >= 2:
                        # nic2[q%2] last read by chunk q-2's final multiplies
                        # (16 tiles back; pool runs ~2 tiles behind, so this
                        # wait is pre-satisfied in steady state)
                        last = [t for t in range((q - 2) * TPQ,
                                                 (q - 1) * TPQ)
                                if s6_pool(t)]
                        if last:
                            dve.wait_ge(s_pool, ptag6[last[-1]])
                    f_ = tag(dve.tensor_scalar(     # nic = (1/s_b)*(-s_t)
                        nic2[q % 2][:], t1_q[:], nst_t[:], None,
                        op0=mybir.AluOpType.mult))
                    dve.wait_ge(s_dve, f_)
                    dve.wait_ge(s_dve, d_)
                    qc_tag[q] = d_
                    qn_tag[q] = f_

                    for tp in range(q * TPQ, (q + 1) * TPQ, 2):
                        pair = (tp, tp + 1)
                        tw = [0, 0]
                        for i, t in enumerate(pair):
                            if y_pool(t):
                                continue
                            if not seen_act[0]:
                                seen_act[0] = True
                                dve.wait_ge(s_act, TA)   # x16 fully written
                            u = t - 4                    # yb slot reuse
                            if u >= 0:
                                if s6_pool(u):
                                    dve.wait_ge(s_pool, ptag6[u])
                            tag_y[t] = tag(dve.tensor_tensor(
                                op=mybir.AluOpType.mult, **y_args(t)))
                        for i, t in enumerate(pair):
                            if y_pool(t):
                                dve.wait_ge(s_pool, ptagy[t])
                            else:
                                dve.wait_ge(s_dve, tag_y[t])
                            tw[i] = tag(dve.tensor_scalar(
                                pb[t % 4][:], yb[t % 4][:].bitcast(u16),
                                M16, None,
                                op0=mybir.AluOpType.bitwise_and))
                        for i, t in enumerate(pair):
                            dve.wait_ge(s_dve, tw[i])
                            tw[i] = tag(dve.tensor_scalar(
                                pb[t % 4][:], pb[t % 4][:], A16, MN16,
                                op0=mybir.AluOpType.add,
                                op1=mybir.AluOpType.max))
                        for i, t in enumerate(pair):
                            dve.wait_ge(s_dve, tw[i])
                            tw[i] = tag(dve.tensor_tensor(   # t -> yb slot
                                yb[t % 4][:], yb[t % 4][:],
                                pb[t % 4][:].bitcast(f16),
                                op=mybir.AluOpType.add))
                        for i, t in enumerate(pair):
                            dve.wait_ge(s_dve, tw[i])
                            tag_nq[t] = tag(dve.tensor_tensor(  # nq -> yb
                                yb[t % 4][:], pb[t % 4][:].bitcast(f16),
                                yb[t % 4][:],
                                op=mybir.AluOpType.subtract))
                        for i, t in enumerate(pair):
                            if s6_pool(t):
                                continue
                            dve.wait_ge(s_dve, tag_nq[t])
                            if t >= n_o:
                                dve.wait_ge(s_ob[t % n_o],
                                            16 * ((t - n_o) // n_o + 1))
                            tag6[t] = tag(dve.tensor_tensor(
                                op=mybir.AluOpType.mult, **s6_args(t)))

            @block.gpsimd
            def _(pool):
                pcnt = 0
                ins = pool.memset(gall[0:1, :], 0.0)
                ins.then_inc(s_pool)
                pcnt += 1
                pool.wait_ge(s_pool, pcnt)
                pool.dma_start(out=cc_warm_in[:, :],
                               in_=gall[0:1, :]).then_inc(s_warm, 16)
                pool.wait_ge(s_warm, 16)
                pool.collective_compute(
                    "AllReduce",
                    mybir.AluOpType.max,
                    replica_groups=[list(range(n_cores))],
                    ins=[cc_warm_in.ap().opt()],
                    outs=[cc_warm_out.ap().opt()],
                ).then_inc(s_cc)
                pool.wait_ge(s_cdma, 16)        # cc_in staged
                pool.collective_compute(
                    "AllReduce",
                    mybir.AluOpType.max,
                    replica_groups=[list(range(n_cores))],
                    ins=[cc_in.ap().opt()],
                    outs=[cc_out.ap().opt()],
                ).then_inc(s_cc)

                def pstep6(t):
                    nonlocal pcnt
                    if (t % TPQ) == 0:
                        pool.wait_ge(s_dve, qn_tag[t // TPQ])
                    pool.wait_ge(s_dve, tag_nq[t])
                    if t >= n_o:
                        pool.wait_ge(s_ob[t % n_o],
                                     16 * ((t - n_o) // n_o + 1))
                    pool.tensor_tensor(
                        op=mybir.AluOpType.mult, **s6_args(t)).then_inc(
                        s_pool)
                    pcnt += 1
                    assert pcnt == ptag6[t]

                for t in range(TB):
                    if y_pool(t):
                        if t == 0:
                            pool.wait_ge(s_act, TA)
                        if (t % TPQ) == 0:
                            pool.wait_ge(s_dve, qc_tag[t // TPQ])
                        u = t - 4
                        if u >= 0 and not s6_pool(u):
                            pool.wait_ge(s_dve, tag6[u])
                        pool.tensor_tensor(
                            op=mybir.AluOpType.mult, **y_args(t)).then_inc(
                            s_pool)
                        pcnt += 1
                        assert pcnt == ptagy[t]
                    if t >= 2 and s6_pool(t - 2):
                        pstep6(t - 2)
                for u in (TB - 2, TB - 1):
                    if s6_pool(u):
                        pstep6(u)

            @block.scalar
            def _(act):
                Copy = mybir.ActivationFunctionType.Copy
                for t in range(TA):
                    act.wait_ge(s_xa[t % n_xa], 16 * (t // n_xa + 1))
                    act.activation(
                        x16[:, t * FA:(t + 1) * FA], xa[t % n_xa][:],
                        Copy).then_inc(s_act)

            @block.sync
            def _(sync):
                # input DMAs: slot free when DVE's reduce AND ACT's copy of
                # the previous occupant are both done
                for t in range(TA):
                    if t >= n_xa:
                        sync.wait_ge(s_dve, tagA[t - n_xa])
                        sync.wait_ge(s_act, t - n_xa + 1)
                    sync.dma_start(
                        out=xa[t % n_xa][:, :],
                        in_=x_ext[:, t * FA:(t + 1) * FA],
                    ).then_inc(s_xa[t % n_xa], 16)
                sync.wait_ge(s_dve, K_mx[0])
                sync.dma_start(out=cc_in[:, :], in_=mx_t[:, :]).then_inc(
                    s_cdma, 16)
                sync.wait_ge(s_cc, 2)
                sync.dma_start(
                    out=gall[:, :],
                    in_=cc_out.ap().broadcast_to([P, 128]),
                ).then_inc(s_cdma, 16)
                for t in range(TB):
                    if s6_pool(t):
                        sync.wait_ge(s_pool, ptag6[t])
                    else:
                        sync.wait_ge(s_dve, tag6[t])
                    sync.dma_start(
                        out=out_ext[:, t * FB:(t + 1) * FB],
                        in_=ob[t % n_o][:, :],
                    ).then_inc(s_ob[t % n_o], 16)
                for i in range(n_o):
                    uses = len([t for t in range(TB) if t % n_o == i])
                    sync.wait_ge(s_ob[i], 16 * uses)

    return nc


_CACHE = {}


def _get_nc():
    if "nc" not in _CACHE:
        _CACHE["nc"] = build_nc()
    return _CACHE["nc"]


def kernel(x: np.ndarray) -> np.ndarray:
    from concourse.bass_utils import run_bass_kernel_spmd

    x = np.asarray(x, dtype=np.float32)
    assert x.shape == FULL_SHAPE
    shards = x.reshape(N_CORES, P, L)
    in_maps = [{"x": np.ascontiguousarray(shards[i])} for i in range(N_CORES)]
    nc = _get_nc()
    res = run_bass_kernel_spmd(nc, in_maps, core_ids=list(range(N_CORES)))
    out = np.stack([r["out"] for r in res.results], axis=0)
    return out.reshape(FULL_SHAPE)
